# revision 1
# baseline (speedup 1.0000x reference)
"""Causal attention head (RoPE) kernel for 8 Trainium2 NeuronCores.

Sharding: 8 cores = 2 batches x 4 head-groups (4 heads each), no
cross-device comms. Per core the device works in feature-major layout:

  - host passes xT = x[b].T (bf16), weights pre-transposed; Wq/Wk rows are
    permuted per head so RoPE even components land in partitions [0:32) and
    odd components in [32:64) of each head's 64-row block.
  - Q^T/K^T/V^T projected with wide 512-row matmuls over 8 c-tiles; RoPE
    applied as new = X*cos - swap(X*sin') (cos is swap-invariant and
    swap(sin') = -sin', so the partition swap runs on the SBUF sin-product
    via 4 block DMAs). V^T is turned into natural-layout V via PE transposes.
  - scores are built transposed (S^T[k,q] = K.Q^T) so softmax'd P^T feeds the
    PV matmul directly (no transposes); the two heads of a pair occupy
    partitions 0-63/64-127 and their K=64 score matmuls run on the two 64x128
    row tiles of the PE array. V carries a ones-column per head, so row 64 of
    the PV output is the softmax denominator for free. exp() runs on ACT with
    the 1/32 scale folded in; no max-subtraction (scores are ~N(0, 0.1)).
  - output O^T is normalized via reciprocal_approx + a K=1 fp16 broadcast
    matmul and written back transposed; the host re-transposes on gather.
"""

import os
import sys
from contextlib import ExitStack

import numpy as np

for _p in ("/opt/trn_rl_repo", "/root/.axon_site/_ro/trn_rl_repo"):
    if os.path.isdir(_p) and _p not in sys.path:
        sys.path.append(_p)

import ml_dtypes

import concourse.bass as bass
import concourse.mybir as mybir
import concourse.tile as tile
from concourse import bacc
from concourse.bass_utils import run_bass_kernel_spmd

P = 128
T = 2048
CIN = 1024
NHC = 4          # heads per core
HS = 64
DOUT = NHC * HS  # 256
NCT = CIN // P   # 8 contraction tiles
SCALE = 1.0 / 32.0  # 1024 ** -0.5

F32 = mybir.dt.float32
BF16 = mybir.dt.bfloat16


def _build_nc():
    nc = bacc.Bacc("TRN2")

    xT = nc.dram_tensor("xT", [CIN, T], BF16, kind="ExternalInput").ap()
    wqT = nc.dram_tensor("wqT", [CIN, DOUT], BF16, kind="ExternalInput").ap()
    wkT = nc.dram_tensor("wkT", [CIN, DOUT], BF16, kind="ExternalInput").ap()
    wvT = nc.dram_tensor("wvT", [CIN, DOUT], BF16, kind="ExternalInput").ap()
    cos4 = nc.dram_tensor("cos4", [P, T], F32, kind="ExternalInput").ap()
    sin4 = nc.dram_tensor("sin4", [P, T], F32, kind="ExternalInput").ap()
    utri = nc.dram_tensor("utri", [P, P], BF16, kind="ExternalInput").ap()
    ident = nc.dram_tensor("ident", [P, P], BF16, kind="ExternalInput").ap()
    outT = nc.dram_tensor("outT", [DOUT, T], F32, kind="ExternalOutput").ap()

    with tile.TileContext(nc) as tc, ExitStack() as ctx:
        const_pool = ctx.enter_context(tc.tile_pool(name="const", bufs=1))
        wpool = ctx.enter_context(tc.tile_pool(name="w", bufs=1))
        qkpool = ctx.enter_context(tc.tile_pool(name="qk", bufs=1))
        vpool = ctx.enter_context(tc.tile_pool(name="vaug", bufs=1))
        phase1 = ExitStack()
        xpool = phase1.enter_context(tc.tile_pool(name="x", bufs=1))
        tmppool = phase1.enter_context(tc.tile_pool(name="tmp", bufs=3))

        # ---- inputs to SBUF (x + weights first: the projection needs them
        # immediately; rope/mask constants are not used until ~40us in)
        w_tiles = {}
        for name, wsrc in (("v", wvT), ("q", wqT), ("k", wkT)):
            w_s = wpool.tile([P, NCT * DOUT], BF16, tag=f"w{name}", name=f"w{name}")
            nc.sync.dma_start(
                w_s.rearrange("p (n d) -> p n d", n=NCT), wsrc.rearrange("(n p) d -> p n d", p=P)
            )
            w_tiles[name] = w_s
        xs = xpool.tile([P, NCT * T], BF16, tag="xs")
        xT_r = xT.rearrange("(n p) t -> p n t", p=P)  # [128, 8, 2048]
        H2 = T // 2
        _dma_engs = (nc.sync, nc.gpsimd, nc.scalar)
        for c in range(NCT):
            for half in range(2):
                eng = _dma_engs[(2 * c + half) % 3]
                eng.dma_start(
                    xs[:, c * T + half * H2: c * T + (half + 1) * H2],
                    xT_r[:, c, half * H2:(half + 1) * H2],
                )
        cos_s = const_pool.tile([P, T], F32, tag="cos")
        nc.sync.dma_start(cos_s[:], cos4)
        sin_s = const_pool.tile([P, T], F32, tag="sin")
        nc.sync.dma_start(sin_s[:], sin4)
        utri_s = const_pool.tile([P, P], BF16, tag="utri")
        nc.sync.dma_start(utri_s[:], utri)
        ident_s = const_pool.tile([P, P], BF16, tag="ident")
        nc.sync.dma_start(ident_s[:], ident)
        ones64 = const_pool.tile([1, HS], mybir.dt.float16, tag="ones64")
        nc.vector.memset(ones64[:], 1.0)

        # ---- phase 1: all three projections as wide 512-row matmuls.
        # V is projected feature-major (V^T) like Q/K to avoid 128 narrow
        # weight-reloading matmuls, then turned into natural-layout V_aug
        # via 32 PE transposes.
        qt = [qkpool.tile([P, T], BF16, tag=f"qt{m}", name=f"qt{m}") for m in range(2)]
        kt = [qkpool.tile([P, T], BF16, tag=f"kt{m}", name=f"kt{m}") for m in range(2)]
        vts = [
            tmppool.tile([P, T], BF16, tag=f"vt{m}", name=f"vt{m}") for m in range(2)
        ]

        with tc.tile_pool(name="pp_proj", bufs=2, space="PSUM") as pp_proj:
            _order = [("v", vts, 0), ("v", vts, 1),
                      ("q", qt, 0), ("k", kt, 0), ("q", qt, 1), ("k", kt, 1)]
            for wname, dst, m in _order:
                w_s = w_tiles[wname]
                if True:
                    ps = pp_proj.tile([P, T], F32, tag="proj")
                    for c in range(NCT):
                        for nch in range(4):
                            sl = slice(nch * 512, (nch + 1) * 512)
                            nc.tensor.matmul(
                                ps[:, sl],
                                lhsT=w_s[:, c * DOUT + m * P: c * DOUT + (m + 1) * P],
                                rhs=xs[:, c * T + nch * 512: c * T + (nch + 1) * 512],
                                start=(c == 0),
                                stop=(c == NCT - 1),
                            )
                    if wname == "v":
                        for nch in range(2):
                            sl = slice(nch * 1024, (nch + 1) * 1024)
                            nc.vector.tensor_copy(dst[m][:, sl], ps[:, sl])
                        continue
                    # RoPE: new = ps*cos - swap(ps*sin'), full-tile granularity
                    # (cos4 is swap-invariant and swap(sin4n) = -sin4n, so the
                    #  partition swap runs on the SBUF sin-product via 4 DMAs)
                    a = tmppool.tile([P, T], F32, tag="ropeA")
                    ap = tmppool.tile([P, T], F32, tag="ropeAp")
                    for nch in range(4):
                        sl = slice(nch * 512, (nch + 1) * 512)
                        nc.vector.tensor_mul(a[:, sl], ps[:, sl], cos_s[:, sl])
                        nc.vector.tensor_mul(ap[:, sl], ps[:, sl], sin_s[:, sl])
                    sw = tmppool.tile([P, T], F32, tag="ropeS")
                    for blk in range(4):
                        s0 = (blk ^ 1) * 32
                        nc.sync.dma_start(
                            sw[blk * 32:(blk + 1) * 32, :], ap[s0:s0 + 32, :]
                        )
                    nc.gpsimd.tensor_sub(dst[m][:], a[:], sw[:])

        # V^T -> natural-layout V_aug tiles (ones column appended per head)
        va = []
        with tc.tile_pool(name="pp_tr", bufs=4, space="PSUM") as pp_tr:
            for t in range(T // P):
                vt = vpool.tile([P, NHC * (HS + 1)], BF16, tag=f"vaug{t}")
                vt_r = vt.rearrange("p (h e) -> p h e", e=HS + 1)
                nc.gpsimd.memset(vt_r[:, :, HS:HS + 1], 1.0)
                for m in range(2):
                    tr = pp_tr.tile([P, P], BF16, tag="tr")
                    nc.tensor.transpose(
                        tr[:], vts[m][:, t * P:(t + 1) * P], ident_s[:]
                    )
                    nc.vector.tensor_copy(
                        vt_r[:, 2 * m:2 * m + 2, 0:HS],
                        tr.rearrange("p (h d) -> p h d", d=HS),
                    )
                va.append(vt)

        phase1.close()  # release xs/tmp zones; PT tiles below reuse them

        # ---- phase 2: attention, two heads interleaved so PE keeps working
        # while ACT runs the other head's exp
        ptpool = ctx.enter_context(tc.tile_pool(name="pt", bufs=1))
        otpool = ctx.enter_context(tc.tile_pool(name="ot", bufs=3))
        rspool = ctx.enter_context(tc.tile_pool(name="rs", bufs=3))
        pp_s = ctx.enter_context(tc.tile_pool(name="pp_s", bufs=3, space="PSUM"))
        pp_o = ctx.enter_context(tc.tile_pool(name="pp_o", bufs=1, space="PSUM"))
        pp_b = ctx.enter_context(tc.tile_pool(name="pp_b", bufs=1, space="PSUM"))

        for pair in ((0, 1), (2, 3)):
            qt_t, kt_t = qt[pair[0] // 2], kt[pair[0] // 2]
            pts = {h: [] for h in pair}
            for j in range(T // P):
                w_j = T - j * P
                ptj_pair = []
                for hi, h in enumerate(pair):
                    ptj = ptpool.tile(
                        [P, w_j], BF16, tag=f"pt{hi}_{j}", name=f"pt{hi}_{j}",
                        bufs=2 if j < 6 else None,
                    )
                    pts[h].append(ptj)
                    ptj_pair.append(ptj)
                for seg0 in range(0, w_j, 1024):
                    seg = min(1024, w_j - seg0)
                    # both heads' score matmuls run concurrently on the two
                    # 64x128 row tiles of the PE array (K=64 each)
                    ps_pair = [
                        pp_s.tile([P, 1024], F32, tag="ps", name=f"ps{hi}")
                        for hi in range(2)
                    ]
                    for s5 in range(0, seg, 512):
                        n = min(512, seg - s5)
                        q0 = j * P + seg0 + s5
                        for hi in range(2):
                            r0 = hi * HS
                            nc.tensor.matmul(
                                ps_pair[hi][:, s5:s5 + n],
                                lhsT=kt_t[r0:r0 + HS, j * P:(j + 1) * P],
                                rhs=qt_t[r0:r0 + HS, q0:q0 + n],
                                start=True,
                                stop=True,
                                tile_position=(hi * HS, 0),
                            )
                    for hi in range(2):
                        nc.scalar.activation(
                            ptj_pair[hi][:, seg0:seg0 + seg],
                            ps_pair[hi][:, 0:seg],
                            mybir.ActivationFunctionType.Exp,
                            scale=SCALE,
                        )
                # causal mask on the diagonal block (col 0 = q-offset j*128)
                for hi in range(2):
                    nc.vector.tensor_mul(
                        ptj_pair[hi][:, 0:P], ptj_pair[hi][:, 0:P], utri_s[:]
                    )

                if j % 4 == 3:
                    qc = j // 4
                    q0 = qc * 512
                    jmax = j
                    for h in pair:
                        po = pp_o.tile([HS + 1, 512], F32, tag="po")
                        # full-width k-tiles first (jj*128 <= q0), partials after
                        order = [jj for jj in range(jmax + 1) if jj * P <= q0]
                        order += [jj for jj in range(jmax + 1) if jj * P > q0]
                        for i, jj in enumerate(order):
                            col0 = max(0, jj * P - q0)
                            nc.tensor.matmul(
                                po[:, col0:512],
                                lhsT=va[jj][:, h * (HS + 1):(h + 1) * (HS + 1)],
                                rhs=pts[h][jj][:, q0 + col0 - jj * P: q0 + 512 - jj * P],
                                start=(i == 0),
                                stop=(i == jmax),
                                skip_group_check=True,
                            )
                        rsum = rspool.tile([1, 512], F32, tag="rsum")
                        nc.vector.tensor_copy(rsum[:], po[HS:HS + 1, :])
                        rs = rspool.tile([1, 512], F32, tag="rs")
                        # approx is ~18 bits — plenty; needs SBUF input (the
                        # bit-twiddled seed reads raw bits, PSUM reads don't)
                        nc.vector.reciprocal_approx_fast(rs[:], rsum[:])
                        rs16 = rspool.tile([1, 512], mybir.dt.float16, tag="rs16")
                        nc.vector.tensor_copy(rs16[:], rs[:])
                        pb = pp_b.tile([HS, 512], F32, tag="pb")
                        nc.tensor.matmul(
                            pb[:],
                            lhsT=ones64[:],
                            rhs=rs16[:],
                            start=True,
                            stop=True,
                        )
                        pbs = otpool.tile([HS, 512], F32, tag="pbs")
                        nc.vector.tensor_copy(pbs[:], pb[:])
                        ot = otpool.tile([HS, 512], F32, tag="ot")
                        nc.vector.tensor_mul(ot[:], po[0:HS, :], pbs[:])
                        nc.sync.dma_start(
                            outT[h * HS:(h + 1) * HS, q0:q0 + 512], ot[:]
                        )
    nc.compile()
    return nc


_CACHE = {}


def _get_nc():
    if "nc" not in _CACHE:
        _CACHE["nc"] = _build_nc()
    return _CACHE["nc"]


def _host_inputs(x, Wq, Wk, Wv):
    bf = ml_dtypes.bfloat16
    B = x.shape[0]
    # RoPE tables (match reference: theta over hs/2 freqs with dim=n_emb)
    i = np.arange(HS // 2, dtype=np.float32)
    theta = np.float32(10000.0) ** (-2.0 * i / np.float32(CIN))
    pos = np.arange(T, dtype=np.float32)
    ang = pos[:, None] * theta[None, :]
    cosT = np.cos(ang).T.astype(np.float32)  # [32, T]
    sinT = np.sin(ang).T.astype(np.float32)
    cos4 = np.ascontiguousarray(np.tile(cosT, (4, 1)))           # [128, T]
    sin4 = np.ascontiguousarray(
        np.tile(np.concatenate([-sinT, sinT], axis=0), (2, 1))
    )  # rows: [-sin, +sin] x2
    utri_np = np.triu(np.ones((P, P), np.float32)).astype(bf)
    ident_np = np.eye(P, dtype=np.float32).astype(bf)

    perm = np.concatenate([np.arange(0, HS, 2), np.arange(1, HS, 2)])
    in_maps = []
    for core in range(8):
        b, g = core // 4, core % 4
        idx = np.concatenate([(4 * g + h) * HS + perm for h in range(NHC)])
        m = {
            "xT": np.ascontiguousarray(x[b].T).astype(bf),
            "wqT": np.ascontiguousarray(Wq[idx].T).astype(bf),
            "wkT": np.ascontiguousarray(Wk[idx].T).astype(bf),
            "wvT": np.ascontiguousarray(Wv[g * DOUT:(g + 1) * DOUT].T).astype(bf),
            "cos4": cos4,
            "sin4": sin4,
            "utri": utri_np,
            "ident": ident_np,
        }
        in_maps.append(m)
    return in_maps


def kernel(x, Wq, Wk, Wv, _trace=False, _trace_kwargs=None):
    x = np.asarray(x)
    Wq, Wk, Wv = np.asarray(Wq), np.asarray(Wk), np.asarray(Wv)
    B = x.shape[0]
    nc = _get_nc()
    in_maps = _host_inputs(x, Wq, Wk, Wv)
    res = run_bass_kernel_spmd(
        nc, in_maps, list(range(8)), trace=_trace, **(_trace_kwargs or {})
    )
    out = np.zeros((B, T, CIN), np.float32)
    for core in range(8):
        b, g = core // 4, core % 4
        out[b, :, g * DOUT:(g + 1) * DOUT] = res.results[core]["outT"].T
    if _trace:
        return out, res
    return out



# revision 15
# speedup vs baseline: 1.1617x; 1.1617x over previous
"""Causal attention head (RoPE) kernel for 8 Trainium2 NeuronCores.

Sharding: 8 cores = 2 batches x 4 head-groups (4 heads each), no
cross-device comms. Per core the device works in feature-major layout:

  - host pre-arranges x and the weights c-tile-major so every input DMA is a
    plain contiguous 2D copy (chunked per c-tile so the first projection
    matmul can start ~12us in); Wq/Wk rows are permuted per head so RoPE
    even components land in partitions [0:32) and odd in [32:64) of each
    head's 64-row block.
  - Q^T/K^T projected with wide 512-col matmuls over 8 c-tiles; RoPE applied
    as new = X*cos - swap(X*sin') with the products cast to bf16 (cos is
    swap-invariant and swap(sin') = -sin', so the partition swap runs on the
    bf16 sin-product via 4 block DMAs on 4 queues); V is projected directly
    in natural layout (x t-tile stationary, Wv moving) with a ones-column
    appended per head so row 64 of the PV output is the softmax denominator.
    The V t-tiles are interleaved into the first score blocks of pair 0 so
    the PE has exp-independent work while the Scalar engine warms up.
  - scores are built transposed (S^T[k,q] = K.Q^T); the two heads of a pair
    write the two 512-col banks of one PSUM tile so a single exp covers both
    heads per 1024 cols (scale 1/32 folded in, no max-subtraction); P^T for
    the pair lives in one SBUF tile (head h at cols [h*w, (h+1)*w)).
  - PV accumulation chains and the reciprocal-broadcast matmuls are delayed
    by one j-block in the PE program order so the PE never waits on exp and
    holds its full-speed p-state; denominators for both heads share one
    reciprocal_approx + fp16 cast; pair 1's first two score blocks are
    interleaved with pair 0's last PV chain to bridge the pair transition.
"""

import os
import sys
from contextlib import ExitStack

import numpy as np

for _p in ("/opt/trn_rl_repo", "/root/.axon_site/_ro/trn_rl_repo"):
    if os.path.isdir(_p) and _p not in sys.path:
        sys.path.append(_p)

import ml_dtypes

import concourse.bass as bass
import concourse.mybir as mybir
import concourse.tile as tile
from concourse import bacc
from concourse.bass_utils import run_bass_kernel_spmd

P = 128
T = 2048
CIN = 1024
NHC = 4          # heads per core
HS = 64
DOUT = NHC * HS  # 256
NCT = CIN // P   # 8 contraction tiles
NTT = T // P     # 16 t/k tiles
SCALE = 1.0 / 32.0  # 1024 ** -0.5

F32 = mybir.dt.float32
BF16 = mybir.dt.bfloat16
FP16 = mybir.dt.float16


def _build_nc():
    nc = bacc.Bacc("TRN2")

    xc = nc.dram_tensor("xc", [P, NCT * T], BF16, kind="ExternalInput").ap()
    wq = nc.dram_tensor("wq", [P, NCT * DOUT], BF16, kind="ExternalInput").ap()
    wk = nc.dram_tensor("wk", [P, NCT * DOUT], BF16, kind="ExternalInput").ap()
    wv = nc.dram_tensor("wv", [P, NCT * DOUT], BF16, kind="ExternalInput").ap()
    cos4 = nc.dram_tensor("cos4", [P, T], BF16, kind="ExternalInput").ap()
    sin4 = nc.dram_tensor("sin4", [P, T], BF16, kind="ExternalInput").ap()
    utri = nc.dram_tensor("utri", [P, P], BF16, kind="ExternalInput").ap()
    outT = nc.dram_tensor("outT", [DOUT, T], F32, kind="ExternalOutput").ap()

    with tile.TileContext(nc) as tc, ExitStack() as ctx:
        const_pool = ctx.enter_context(tc.tile_pool(name="const", bufs=1))
        wpool = ctx.enter_context(tc.tile_pool(name="w", bufs=1))
        qkpool = ctx.enter_context(tc.tile_pool(name="qk", bufs=1))
        vpool = ctx.enter_context(tc.tile_pool(name="vaug", bufs=1))
        ptpool = ctx.enter_context(tc.tile_pool(name="pt", bufs=1))
        otpool = ctx.enter_context(tc.tile_pool(name="ot", bufs=2))
        rspool = ctx.enter_context(tc.tile_pool(name="rs", bufs=1))
        phase1 = ExitStack()
        xpool = phase1.enter_context(tc.tile_pool(name="x", bufs=1))
        tmppool = phase1.enter_context(tc.tile_pool(name="tmp", bufs=1))

        # ---- inputs to SBUF.  Weight/x DMAs lead on each queue so the first
        # projection matmul starts as early as possible; rope/mask constants
        # follow (not needed until ~15us in).
        w_tiles = {}
        xs = xpool.tile([P, NCT * T], BF16, tag="xs")
        for eng, (name, wsrc) in zip(
            (nc.sync, nc.gpsimd, nc.scalar), (("q", wq), ("k", wk), ("v", wv))
        ):
            w_s = wpool.tile([P, NCT * DOUT], BF16, tag=f"w{name}", name=f"w{name}")
            eng.dma_start(w_s[:], wsrc)
            w_tiles[name] = w_s
        _engs = (nc.gpsimd, nc.scalar, nc.sync)
        for c in range(NCT):
            _engs[c % 3].dma_start(
                xs[:, c * T:(c + 1) * T], xc[:, c * T:(c + 1) * T]
            )
        cos_s = const_pool.tile([P, T], BF16, tag="cos")
        nc.scalar.dma_start(cos_s[:], cos4)
        sin_s = const_pool.tile([P, T], BF16, tag="sin")
        nc.gpsimd.dma_start(sin_s[:], sin4)
        utri_s = const_pool.tile([P, P], BF16, tag="utri")
        nc.sync.dma_start(utri_s[:], utri)
        ones64 = const_pool.tile([1, HS], FP16, tag="ones64")
        nc.vector.memset(ones64[:], 1.0)

        # ---- phase 1a: Q^T/K^T projections + RoPE.
        qt = [qkpool.tile([P, T], BF16, tag=f"qt{m}", name=f"qt{m}") for m in range(2)]
        kt = [qkpool.tile([P, T], BF16, tag=f"kt{m}", name=f"kt{m}") for m in range(2)]

        with tc.tile_pool(name="pp_proj", bufs=2, space="PSUM") as pp_proj:
            for wname, dst, m in (
                ("q", qt, 0), ("k", kt, 0), ("q", qt, 1), ("k", kt, 1)
            ):
                w_s = w_tiles[wname]
                ps = pp_proj.tile([P, T], F32, tag="proj")
                for c in range(NCT):
                    for nch in range(4):
                        sl = slice(nch * 512, (nch + 1) * 512)
                        nc.tensor.matmul(
                            ps[:, sl],
                            lhsT=w_s[:, c * DOUT + m * P: c * DOUT + (m + 1) * P],
                            rhs=xs[:, c * T + nch * 512: c * T + (nch + 1) * 512],
                            start=(c == 0),
                            stop=(c == NCT - 1),
                        )
                a = tmppool.tile([P, T], BF16, tag="ropeA")
                apr = tmppool.tile([P, T], BF16, tag="ropeAp")
                nc.vector.tensor_mul(a[:], ps[:], cos_s[:])
                nc.vector.tensor_mul(apr[:], ps[:], sin_s[:])
                sw = tmppool.tile([P, T], BF16, tag="ropeS")
                for blk in range(4):
                    s0 = (blk ^ 1) * 32
                    _engs[blk % 3].dma_start(
                        sw[blk * 32:(blk + 1) * 32, :], apr[s0:s0 + 32, :]
                    )
                nc.gpsimd.tensor_sub(dst[m][:], a[:], sw[:])

        # ---- phase 1b/2 shared machinery
        w_v = w_tiles["v"]
        va = []
        pp_s = ctx.enter_context(tc.tile_pool(name="pp_s", bufs=2, space="PSUM"))
        pp_ob = {}  # pp_o/pp_b created after phase1's PSUM pool closes
        pp_v = phase1.enter_context(tc.tile_pool(name="pp_v", bufs=4, space="PSUM"))

        def v_tile(t):
            """Project V t-tile in natural layout, append ones column."""
            vt = vpool.tile([P, NHC * (HS + 1)], BF16, tag=f"vaug{t}")
            vt_r = vt.rearrange("p (h e) -> p h e", e=HS + 1)
            nc.gpsimd.memset(vt_r[:, :, HS:HS + 1], 1.0)
            vp = pp_v.tile([P, DOUT], F32, tag="vp")
            for c in range(NCT):
                nc.tensor.matmul(
                    vp[:],
                    lhsT=xs[:, c * T + t * P: c * T + (t + 1) * P],
                    rhs=w_v[:, c * DOUT:(c + 1) * DOUT],
                    start=(c == 0),
                    stop=(c == NCT - 1),
                )
            nc.vector.tensor_copy(
                vt_r[:, :, 0:HS], vp.rearrange("p (h d) -> p h d", d=HS)
            )
            va.append(vt)


        def scores_j(pair, j, pts):
            """Score matmuls + exp + diag mask for k-block j of a pair."""
            qt_t, kt_t = qt[pair[0] // 2], kt[pair[0] // 2]
            w_j = T - j * P
            ptj = ptpool.tile(
                [P, 2 * w_j], BF16, tag=f"pt{j}", name=f"pt{j}",
                bufs=2 if j < 1 else None,
            )
            pts.append(ptj)
            pt_r = ptj.rearrange("p (h w) -> p h w", h=2)
            for s in range(0, w_j, 512):
                n = min(512, w_j - s)
                ps = pp_s.tile([P, 1024], F32, tag="ps", name="ps")
                for hi in range(2):
                    r0 = hi * HS
                    nc.tensor.matmul(
                        ps[:, hi * 512: hi * 512 + n],
                        lhsT=kt_t[r0:r0 + HS, j * P:(j + 1) * P],
                        rhs=qt_t[r0:r0 + HS, j * P + s: j * P + s + n],
                        start=True,
                        stop=True,
                        tile_position=(r0, 0),
                    )
                nc.scalar.activation(
                    pt_r[:, :, s:s + n],
                    ps.rearrange("p (h c) -> p h c", h=2)[:, :, 0:n],
                    mybir.ActivationFunctionType.Exp,
                    scale=SCALE,
                )
            # causal mask on the diagonal block (col 0 = q-offset j*128)
            for hi in range(2):
                nc.vector.tensor_mul(
                    ptj[:, hi * w_j: hi * w_j + P],
                    ptj[:, hi * w_j: hi * w_j + P],
                    utri_s[:],
                )

        def pv_chunk(pair, qc, pts, norm_q):
            """PV accumulation chains for q-chunk qc (both heads)."""
            q0 = qc * 512
            jmax = 4 * qc + 3
            pos = []
            for hi, h in enumerate(pair):
                po = pp_ob["o"].tile([HS + 1, 512], F32, tag="po", name=f"po{hi}")
                order = [jj for jj in range(jmax + 1) if jj * P <= q0]
                order += [jj for jj in range(jmax + 1) if jj * P > q0]
                for i, jj in enumerate(order):
                    col0 = max(0, jj * P - q0)
                    w_jj = T - jj * P
                    qoff = q0 + col0 - jj * P
                    nc.tensor.matmul(
                        po[:, col0:512],
                        lhsT=va[jj][:, h * (HS + 1):(h + 1) * (HS + 1)],
                        rhs=pts[jj][:, hi * w_jj + qoff: hi * w_jj + qoff + 512 - col0],
                        start=(i == 0),
                        stop=(i == jmax),
                        skip_group_check=True,
                    )
                pos.append(po)
            # denominators for both heads (rows 0 and 64 so partition bases
            # stay aligned) -> one reciprocal + two fp16 casts
            dn = rspool.tile([HS + 1, 512], F32, tag="dn")
            for hi in range(2):
                nc.vector.tensor_copy(
                    dn[hi * HS:hi * HS + 1, :], pos[hi][HS:HS + 1, :]
                )
            rs = rspool.tile([HS + 1, 512], F32, tag="rs")
            nc.vector.reciprocal_approx_fast(rs[:], dn[:])
            rs16 = [
                rspool.tile([1, 512], FP16, tag=f"rs16_{hi}", name=f"rs16_{hi}")
                for hi in range(2)
            ]
            for hi in range(2):
                nc.vector.tensor_copy(rs16[hi][:], rs[hi * HS:hi * HS + 1, :])
            norm_q.append((qc, pos, rs16))

        def norm_chunk(pair, item):
            """Broadcast 1/denom via K=1 matmul and write the chunk out."""
            qc, pos, rs16 = item
            q0 = qc * 512
            for hi, h in enumerate(pair):
                poS = otpool.tile([HS, 512], F32, tag="poS", name=f"poS{hi}")
                nc.scalar.activation(
                    poS[:], pos[hi][0:HS, :], mybir.ActivationFunctionType.Copy
                )
                pb = pp_ob["b"].tile([HS, 512], F32, tag="pb", name=f"pb{hi}")
                nc.tensor.matmul(
                    pb[:],
                    lhsT=ones64[:],
                    rhs=rs16[hi][:],
                    start=True,
                    stop=True,
                )
                ot = otpool.tile([HS, 512], F32, tag="ot")
                nc.vector.tensor_mul(ot[:], poS[:], pb[:])
                (nc.sync, nc.gpsimd)[hi].dma_start(
                    outT[h * HS:(h + 1) * HS, q0:q0 + 512], ot[:]
                )

        # ---- phase 2 schedule
        pairs = ((0, 1), (2, 3))
        all_pts = {0: [], 1: []}
        for pi, pair in enumerate(pairs):
            pts = all_pts[pi]
            norm_q = []
            for j in range(len(pts), NTT):
                scores_j(pair, j, pts)
                if pi == 0 and j < 4:
                    for t in range(4 * j, 4 * j + 4):
                        v_tile(t)
                    if j == 3:
                        phase1.close()  # xs/tmp zones free; pt reuses them
                        pp_ob["o"] = ctx.enter_context(
                            tc.tile_pool(name="pp_o", bufs=2, space="PSUM")
                        )
                        pp_ob["b"] = ctx.enter_context(
                            tc.tile_pool(name="pp_b", bufs=2, space="PSUM")
                        )
                if j % 4 == 0 and j > 0:
                    pv_chunk(pair, j // 4 - 1, pts, norm_q)
                elif j % 4 == 1 and norm_q:
                    norm_chunk(pair, norm_q.pop(0))
            if pi == 0:
                # bridge the pair transition: pair 1's first score blocks
                # give the PE exp-independent work during pair 0's drain
                scores_j(pairs[1], 0, all_pts[1])
                pv_chunk(pair, 3, pts, norm_q)
                scores_j(pairs[1], 1, all_pts[1])
                norm_chunk(pair, norm_q.pop(0))
            else:
                pv_chunk(pair, 3, pts, norm_q)
                norm_chunk(pair, norm_q.pop(0))
    nc.compile()
    return nc


_CACHE = {}


def _get_nc():
    if "nc" not in _CACHE:
        _CACHE["nc"] = _build_nc()
    return _CACHE["nc"]


def _host_inputs(x, Wq, Wk, Wv):
    bf = ml_dtypes.bfloat16
    # RoPE tables (match reference: theta over hs/2 freqs with dim=n_emb)
    i = np.arange(HS // 2, dtype=np.float32)
    theta = np.float32(10000.0) ** (-2.0 * i / np.float32(CIN))
    pos = np.arange(T, dtype=np.float32)
    ang = pos[:, None] * theta[None, :]
    cosT = np.cos(ang).T.astype(np.float32)  # [32, T]
    sinT = np.sin(ang).T.astype(np.float32)
    cos4 = np.ascontiguousarray(np.tile(cosT, (4, 1))).astype(bf)  # [128, T]
    sin4 = np.ascontiguousarray(
        np.tile(np.concatenate([-sinT, sinT], axis=0), (2, 1))
    ).astype(bf)  # rows: [-sin, +sin] x2
    utri_np = np.triu(np.ones((P, P), np.float32)).astype(bf)

    def cmajor(w):  # [256 out rows, 1024 in] -> [128, 8*256] c-tile-major
        return np.ascontiguousarray(
            w.T.reshape(NCT, P, DOUT).transpose(1, 0, 2).reshape(P, NCT * DOUT)
        ).astype(bf)

    perm = np.concatenate([np.arange(0, HS, 2), np.arange(1, HS, 2)])
    in_maps = []
    for core in range(8):
        b, g = core // 4, core % 4
        idx = np.concatenate([(4 * g + h) * HS + perm for h in range(NHC)])
        xb = np.ascontiguousarray(
            x[b].T.reshape(NCT, P, T).transpose(1, 0, 2).reshape(P, NCT * T)
        ).astype(bf)
        m = {
            "xc": xb,
            "wq": cmajor(Wq[idx]),
            "wk": cmajor(Wk[idx]),
            "wv": cmajor(Wv[g * DOUT:(g + 1) * DOUT]),
            "cos4": cos4,
            "sin4": sin4,
            "utri": utri_np,
        }
        in_maps.append(m)
    return in_maps


def kernel(x, Wq, Wk, Wv, _trace=False, _trace_kwargs=None):
    x = np.asarray(x)
    Wq, Wk, Wv = np.asarray(Wq), np.asarray(Wk), np.asarray(Wv)
    B = x.shape[0]
    nc = _get_nc()
    in_maps = _host_inputs(x, Wq, Wk, Wv)
    res = run_bass_kernel_spmd(
        nc, in_maps, list(range(8)), trace=_trace, **(_trace_kwargs or {})
    )
    out = np.zeros((B, T, CIN), np.float32)
    for core in range(8):
        b, g = core // 4, core % 4
        out[b, :, g * DOUT:(g + 1) * DOUT] = res.results[core]["outT"].T
    if _trace:
        return out, res
    return out


# revision 20
# speedup vs baseline: 1.2308x; 1.0594x over previous
"""Causal attention head (RoPE) kernel for 8 Trainium2 NeuronCores.

Sharding: 8 cores = 2 batches x 4 head-groups (4 heads each), no
cross-device comms. Per core the device works in feature-major layout:

  - host pre-arranges x and the weights c-tile-major so every input DMA is a
    plain contiguous 2D copy (chunked per c-tile so the first projection
    matmul can start ~12us in); Wq/Wk rows are permuted per head so RoPE
    even components land in partitions [0:32) and odd in [32:64) of each
    head's 64-row block.
  - Q^T/K^T projected with wide 512-col matmuls over 8 c-tiles; RoPE applied
    as new = X*cos - swap(X*sin') with the products cast to bf16 (cos is
    swap-invariant and swap(sin') = -sin', so the partition swap runs on the
    bf16 sin-product via 4 block DMAs on 4 queues); V is projected directly
    in natural layout (x t-tile stationary, Wv moving) with a ones-column
    appended per head so row 64 of the PV output is the softmax denominator.
    The V t-tiles are interleaved into the first score blocks of pair 0 so
    the PE has exp-independent work while the Scalar engine warms up.
  - scores are built transposed (S^T[k,q] = K.Q^T); the two heads of a pair
    write the two 512-col banks of one PSUM tile so a single exp covers both
    heads per 1024 cols (scale 1/32 folded in, no max-subtraction); P^T for
    the pair lives in one SBUF tile (head h at cols [h*w, (h+1)*w)).
  - PV accumulation chains and the reciprocal-broadcast matmuls are delayed
    by one j-block in the PE program order so the PE never waits on exp and
    holds its full-speed p-state; denominators for both heads share one
    reciprocal_approx + fp16 cast; pair 1's first two score blocks are
    interleaved with pair 0's last PV chain to bridge the pair transition.
"""

import os
import sys
from contextlib import ExitStack

import numpy as np

for _p in ("/opt/trn_rl_repo", "/root/.axon_site/_ro/trn_rl_repo"):
    if os.path.isdir(_p) and _p not in sys.path:
        sys.path.append(_p)

import ml_dtypes

import concourse.bass as bass
import concourse.mybir as mybir
import concourse.tile as tile
from concourse import bacc
from concourse.bass_utils import run_bass_kernel_spmd

P = 128
T = 2048
CIN = 1024
NHC = 4          # heads per core
HS = 64
DOUT = NHC * HS  # 256
NCT = CIN // P   # 8 contraction tiles
NTT = T // P     # 16 t/k tiles
SCALE = 1.0 / 32.0  # 1024 ** -0.5

F32 = mybir.dt.float32
BF16 = mybir.dt.bfloat16
FP16 = mybir.dt.float16


def _build_nc():
    nc = bacc.Bacc("TRN2")

    xc = nc.dram_tensor("xc", [P, NCT * T], BF16, kind="ExternalInput").ap()
    wq = nc.dram_tensor("wq", [P, NCT * DOUT], BF16, kind="ExternalInput").ap()
    wk = nc.dram_tensor("wk", [P, NCT * DOUT], BF16, kind="ExternalInput").ap()
    wv = nc.dram_tensor("wv", [P, NCT * DOUT], BF16, kind="ExternalInput").ap()
    cos4 = nc.dram_tensor("cos4", [P, T], BF16, kind="ExternalInput").ap()
    sin4 = nc.dram_tensor("sin4", [P, T], BF16, kind="ExternalInput").ap()
    utri = nc.dram_tensor("utri", [P, P], BF16, kind="ExternalInput").ap()
    pswap = nc.dram_tensor("pswap", [P, P], BF16, kind="ExternalInput").ap()
    outT = nc.dram_tensor("outT", [DOUT, T], F32, kind="ExternalOutput").ap()

    with tile.TileContext(nc) as tc, ExitStack() as ctx:
        const_pool = ctx.enter_context(tc.tile_pool(name="const", bufs=1))
        wpool = ctx.enter_context(tc.tile_pool(name="w", bufs=1))
        qkpool = ctx.enter_context(tc.tile_pool(name="qk", bufs=1))
        vpool = ctx.enter_context(tc.tile_pool(name="vaug", bufs=1))
        ptpool = ctx.enter_context(tc.tile_pool(name="pt", bufs=1))
        otpool = ctx.enter_context(tc.tile_pool(name="ot", bufs=2))
        rspool = ctx.enter_context(tc.tile_pool(name="rs", bufs=1))
        phase1 = ExitStack()
        xpool = phase1.enter_context(tc.tile_pool(name="x", bufs=1))
        tmppool = phase1.enter_context(tc.tile_pool(name="tmp", bufs=1))

        # ---- inputs to SBUF.  DMAs are issued in consumption order (queue
        # descriptors drain roughly FIFO across the ring): wq + x c-tiles
        # first, then wk, then the late-needed wv / rope / mask constants.
        w_tiles = {}
        for name, wsrc in (("q", wq), ("k", wk), ("v", wv)):
            w_tiles[name] = wpool.tile(
                [P, NCT * DOUT], BF16, tag=f"w{name}", name=f"w{name}"
            )
        xs = xpool.tile([P, NCT * T], BF16, tag="xs")
        cos_s = const_pool.tile([P, T], BF16, tag="cos")
        sin_s = const_pool.tile([P, T], BF16, tag="sin")
        utri_s = const_pool.tile([P, P], BF16, tag="utri")
        pswap_s = const_pool.tile([P, P], BF16, tag="pswap")
        dmas = [(w_tiles["q"][:], wq)]
        dmas += [
            (xs[:, c * T:(c + 1) * T], xc[:, c * T:(c + 1) * T]) for c in range(NCT)
        ]
        dmas.insert(3, (w_tiles["k"][:], wk))
        dmas += [
            (w_tiles["v"][:], wv), (cos_s[:], cos4), (sin_s[:], sin4),
            (pswap_s[:], pswap), (utri_s[:], utri),
        ]
        _engs = (nc.sync, nc.gpsimd, nc.scalar)
        for i, (dst, src) in enumerate(dmas):
            _engs[i % 3].dma_start(dst, src)
        ones64 = const_pool.tile([1, HS], FP16, tag="ones64")
        nc.vector.memset(ones64[:], 1.0)

        # ---- phase 1a: Q^T/K^T projections + RoPE, 512-col chunks.
        # new = ps*cos - swap(ps*sin'); the partition swap runs on the PE as
        # a permutation matmul on the bf16 sin-product, and the subtract on
        # DVE (one PSUM operand).  Swap matmuls trail the projection chunks
        # by one step so the PE never waits on the DVE multiplies.
        qt = [qkpool.tile([P, T], BF16, tag=f"qt{m}", name=f"qt{m}") for m in range(2)]
        kt = [qkpool.tile([P, T], BF16, tag=f"kt{m}", name=f"kt{m}") for m in range(2)]

        with tc.tile_pool(name="pp_proj", bufs=3, space="PSUM") as pp_proj, \
                tc.tile_pool(name="pp_sw", bufs=2, space="PSUM") as pp_sw:
            pending = []

            def flush_swap():
                while pending:
                    dst_sl, apr_p = pending.pop(0)
                    swp = pp_sw.tile([P, 512], F32, tag="swp")
                    nc.tensor.matmul(
                        swp[:], lhsT=pswap_s[:], rhs=apr_p[:],
                        start=True, stop=True,
                    )
                    nc.vector.tensor_sub(dst_sl, aprev.pop(0)[:], swp[:])

            aprev = []
            for wname, dst, m in (
                ("q", qt, 0), ("k", kt, 0), ("q", qt, 1), ("k", kt, 1)
            ):
                w_s = w_tiles[wname]
                for nch in range(4):
                    sl = slice(nch * 512, (nch + 1) * 512)
                    ps = pp_proj.tile([P, 512], F32, tag="proj")
                    for c in range(NCT):
                        nc.tensor.matmul(
                            ps[:],
                            lhsT=w_s[:, c * DOUT + m * P: c * DOUT + (m + 1) * P],
                            rhs=xs[:, c * T + nch * 512: c * T + (nch + 1) * 512],
                            start=(c == 0),
                            stop=(c == NCT - 1),
                        )
                    a = tmppool.tile([P, 512], BF16, tag="ropeA", bufs=3)
                    apr = tmppool.tile([P, 512], BF16, tag="ropeAp", bufs=3)
                    nc.vector.tensor_mul(a[:], ps[:], cos_s[:, sl])
                    nc.vector.tensor_mul(apr[:], ps[:], sin_s[:, sl])
                    flush_swap()
                    aprev.append(a)
                    pending.append((dst[m][:, sl], apr))
            flush_swap()

        # ---- phase 1b/2 shared machinery
        w_v = w_tiles["v"]
        va = []
        pp_s = ctx.enter_context(tc.tile_pool(name="pp_s", bufs=2, space="PSUM"))
        pp_ob = {}  # pp_o/pp_b created after phase1's PSUM pool closes
        pp_v = phase1.enter_context(tc.tile_pool(name="pp_v", bufs=4, space="PSUM"))

        def v_tile(t):
            """Project V t-tile in natural layout, append ones column."""
            vt = vpool.tile([P, NHC * (HS + 1)], BF16, tag=f"vaug{t}")
            vt_r = vt.rearrange("p (h e) -> p h e", e=HS + 1)
            nc.gpsimd.memset(vt_r[:, :, HS:HS + 1], 1.0)
            vp = pp_v.tile([P, DOUT], F32, tag="vp")
            for c in range(NCT):
                nc.tensor.matmul(
                    vp[:],
                    lhsT=xs[:, c * T + t * P: c * T + (t + 1) * P],
                    rhs=w_v[:, c * DOUT:(c + 1) * DOUT],
                    start=(c == 0),
                    stop=(c == NCT - 1),
                )
            nc.vector.tensor_copy(
                vt_r[:, :, 0:HS], vp.rearrange("p (h d) -> p h d", d=HS)
            )
            va.append(vt)


        def scores_j(pair, j, pts):
            """Score matmuls + exp + diag mask for k-block j of a pair."""
            qt_t, kt_t = qt[pair[0] // 2], kt[pair[0] // 2]
            w_j = T - j * P
            ptj = ptpool.tile(
                [P, 2 * w_j], BF16, tag=f"pt{j}", name=f"pt{j}",
                bufs=2 if j < 1 else None,
            )
            pts.append(ptj)
            pt_r = ptj.rearrange("p (h w) -> p h w", h=2)
            for s in range(0, w_j, 512):
                n = min(512, w_j - s)
                ps = pp_s.tile([P, 1024], F32, tag="ps", name="ps")
                for hi in range(2):
                    r0 = hi * HS
                    nc.tensor.matmul(
                        ps[:, hi * 512: hi * 512 + n],
                        lhsT=kt_t[r0:r0 + HS, j * P:(j + 1) * P],
                        rhs=qt_t[r0:r0 + HS, j * P + s: j * P + s + n],
                        start=True,
                        stop=True,
                        tile_position=(r0, 0),
                    )
                nc.scalar.activation(
                    pt_r[:, :, s:s + n],
                    ps.rearrange("p (h c) -> p h c", h=2)[:, :, 0:n],
                    mybir.ActivationFunctionType.Exp,
                    scale=SCALE,
                )
            # causal mask on the diagonal block (col 0 = q-offset j*128)
            for hi in range(2):
                nc.vector.tensor_mul(
                    ptj[:, hi * w_j: hi * w_j + P],
                    ptj[:, hi * w_j: hi * w_j + P],
                    utri_s[:],
                )

        def pv_chunk(pair, qc, pts, norm_q):
            """PV accumulation chains for q-chunk qc (both heads)."""
            q0 = qc * 512
            jmax = 4 * qc + 3
            pos = []
            for hi, h in enumerate(pair):
                po = pp_ob["o"].tile([HS + 1, 512], F32, tag="po", name=f"po{hi}")
                order = [jj for jj in range(jmax + 1) if jj * P <= q0]
                order += [jj for jj in range(jmax + 1) if jj * P > q0]
                for i, jj in enumerate(order):
                    col0 = max(0, jj * P - q0)
                    w_jj = T - jj * P
                    qoff = q0 + col0 - jj * P
                    nc.tensor.matmul(
                        po[:, col0:512],
                        lhsT=va[jj][:, h * (HS + 1):(h + 1) * (HS + 1)],
                        rhs=pts[jj][:, hi * w_jj + qoff: hi * w_jj + qoff + 512 - col0],
                        start=(i == 0),
                        stop=(i == jmax),
                        skip_group_check=True,
                    )
                pos.append(po)
            # denominators for both heads (rows 0 and 64 so partition bases
            # stay aligned) -> one reciprocal + two fp16 casts
            dn = rspool.tile([HS + 1, 512], F32, tag="dn")
            for hi in range(2):
                nc.vector.tensor_copy(
                    dn[hi * HS:hi * HS + 1, :], pos[hi][HS:HS + 1, :]
                )
            rs = rspool.tile([HS + 1, 512], F32, tag="rs")
            nc.vector.reciprocal_approx_fast(rs[:], dn[:])
            rs16 = [
                rspool.tile([1, 512], FP16, tag=f"rs16_{hi}", name=f"rs16_{hi}")
                for hi in range(2)
            ]
            for hi in range(2):
                nc.vector.tensor_copy(rs16[hi][:], rs[hi * HS:hi * HS + 1, :])
            norm_q.append((qc, pos, rs16))

        def norm_chunk(pair, item):
            """Broadcast 1/denom via K=1 matmul and write the chunk out."""
            qc, pos, rs16 = item
            q0 = qc * 512
            for hi, h in enumerate(pair):
                poS = otpool.tile([HS, 512], F32, tag="poS", name=f"poS{hi}")
                nc.scalar.activation(
                    poS[:], pos[hi][0:HS, :], mybir.ActivationFunctionType.Copy
                )
                pb = pp_ob["b"].tile([HS, 512], F32, tag="pb", name=f"pb{hi}")
                nc.tensor.matmul(
                    pb[:],
                    lhsT=ones64[:],
                    rhs=rs16[hi][:],
                    start=True,
                    stop=True,
                )
                ot = otpool.tile([HS, 512], F32, tag="ot")
                nc.vector.tensor_mul(ot[:], poS[:], pb[:])
                (nc.sync, nc.gpsimd)[hi].dma_start(
                    outT[h * HS:(h + 1) * HS, q0:q0 + 512], ot[:]
                )

        # ---- phase 2 schedule
        pairs = ((0, 1), (2, 3))
        all_pts = {0: [], 1: []}
        for pi, pair in enumerate(pairs):
            pts = all_pts[pi]
            norm_q = []
            for j in range(len(pts), NTT):
                if pi == 0 and j < 4:
                    for t in range(4 * j, 4 * j + 4):
                        v_tile(t)
                scores_j(pair, j, pts)
                if pi == 0 and j < 4:
                    if j == 3:
                        phase1.close()  # xs/tmp zones free; pt reuses them
                        pp_ob["o"] = ctx.enter_context(
                            tc.tile_pool(name="pp_o", bufs=2, space="PSUM")
                        )
                        pp_ob["b"] = ctx.enter_context(
                            tc.tile_pool(name="pp_b", bufs=2, space="PSUM")
                        )
                if j % 4 == 0 and j > 0:
                    pv_chunk(pair, j // 4 - 1, pts, norm_q)
                elif j % 4 == 1 and norm_q:
                    norm_chunk(pair, norm_q.pop(0))
            if pi == 0:
                # bridge the pair transition: pair 1's first score blocks
                # give the PE exp-independent work during pair 0's drain
                scores_j(pairs[1], 0, all_pts[1])
                pv_chunk(pair, 3, pts, norm_q)
                scores_j(pairs[1], 1, all_pts[1])
                norm_chunk(pair, norm_q.pop(0))
            else:
                pv_chunk(pair, 3, pts, norm_q)
                norm_chunk(pair, norm_q.pop(0))
    nc.compile()
    return nc


_CACHE = {}


def _get_nc():
    if "nc" not in _CACHE:
        _CACHE["nc"] = _build_nc()
    return _CACHE["nc"]


def _host_inputs(x, Wq, Wk, Wv):
    bf = ml_dtypes.bfloat16
    # RoPE tables (match reference: theta over hs/2 freqs with dim=n_emb)
    i = np.arange(HS // 2, dtype=np.float32)
    theta = np.float32(10000.0) ** (-2.0 * i / np.float32(CIN))
    pos = np.arange(T, dtype=np.float32)
    ang = pos[:, None] * theta[None, :]
    cosT = np.cos(ang).T.astype(np.float32)  # [32, T]
    sinT = np.sin(ang).T.astype(np.float32)
    cos4 = np.ascontiguousarray(np.tile(cosT, (4, 1))).astype(bf)  # [128, T]
    sin4 = np.ascontiguousarray(
        np.tile(np.concatenate([-sinT, sinT], axis=0), (2, 1))
    ).astype(bf)  # rows: [-sin, +sin] x2
    utri_np = np.triu(np.ones((P, P), np.float32)).astype(bf)
    pswap_np = np.zeros((P, P), np.float32)
    pswap_np[np.arange(P), np.arange(P) ^ 32] = 1.0
    pswap_np = pswap_np.astype(bf)

    def cmajor(w):  # [256 out rows, 1024 in] -> [128, 8*256] c-tile-major
        return np.ascontiguousarray(
            w.T.reshape(NCT, P, DOUT).transpose(1, 0, 2).reshape(P, NCT * DOUT)
        ).astype(bf)

    perm = np.concatenate([np.arange(0, HS, 2), np.arange(1, HS, 2)])
    in_maps = []
    for core in range(8):
        b, g = core // 4, core % 4
        idx = np.concatenate([(4 * g + h) * HS + perm for h in range(NHC)])
        xb = np.ascontiguousarray(
            x[b].T.reshape(NCT, P, T).transpose(1, 0, 2).reshape(P, NCT * T)
        ).astype(bf)
        m = {
            "xc": xb,
            "wq": cmajor(Wq[idx]),
            "wk": cmajor(Wk[idx]),
            "wv": cmajor(Wv[g * DOUT:(g + 1) * DOUT]),
            "cos4": cos4,
            "sin4": sin4,
            "utri": utri_np,
            "pswap": pswap_np,
        }
        in_maps.append(m)
    return in_maps


def kernel(x, Wq, Wk, Wv, _trace=False, _trace_kwargs=None):
    x = np.asarray(x)
    Wq, Wk, Wv = np.asarray(Wq), np.asarray(Wk), np.asarray(Wv)
    B = x.shape[0]
    nc = _get_nc()
    in_maps = _host_inputs(x, Wq, Wk, Wv)
    res = run_bass_kernel_spmd(
        nc, in_maps, list(range(8)), trace=_trace, **(_trace_kwargs or {})
    )
    out = np.zeros((B, T, CIN), np.float32)
    for core in range(8):
        b, g = core // 4, core % 4
        out[b, :, g * DOUT:(g + 1) * DOUT] = res.results[core]["outT"].T
    if _trace:
        return out, res
    return out


# revision 21
# speedup vs baseline: 1.2890x; 1.0473x over previous
"""Causal attention head (RoPE) kernel for 8 Trainium2 NeuronCores.

Sharding: 8 cores = 2 batches x 4 head-groups (4 heads each), no
cross-device comms. Per core the device works in feature-major layout:

  - host pre-arranges x and the weights c-tile-major so every input DMA is a
    plain contiguous 2D copy (chunked per c-tile so the first projection
    matmul can start ~12us in); Wq/Wk rows are permuted per head so RoPE
    even components land in partitions [0:32) and odd in [32:64) of each
    head's 64-row block.
  - Q^T/K^T projected with wide 512-col matmuls over 8 c-tiles; RoPE applied
    as new = X*cos - swap(X*sin') with the products cast to bf16 (cos is
    swap-invariant and swap(sin') = -sin', so the partition swap runs on the
    bf16 sin-product via 4 block DMAs on 4 queues); V is projected directly
    in natural layout (x t-tile stationary, Wv moving) with a ones-column
    appended per head so row 64 of the PV output is the softmax denominator.
    The V t-tiles are interleaved into the first score blocks of pair 0 so
    the PE has exp-independent work while the Scalar engine warms up.
  - scores are built transposed (S^T[k,q] = K.Q^T); the two heads of a pair
    write the two 512-col banks of one PSUM tile so a single exp covers both
    heads per 1024 cols (scale 1/32 folded in, no max-subtraction); P^T for
    the pair lives in one SBUF tile (head h at cols [h*w, (h+1)*w)).
  - PV accumulation chains and the reciprocal-broadcast matmuls are delayed
    by one j-block in the PE program order so the PE never waits on exp and
    holds its full-speed p-state; denominators for both heads share one
    reciprocal_approx + fp16 cast; pair 1's first two score blocks are
    interleaved with pair 0's last PV chain to bridge the pair transition.
"""

import os
import sys
from contextlib import ExitStack

import numpy as np

for _p in ("/opt/trn_rl_repo", "/root/.axon_site/_ro/trn_rl_repo"):
    if os.path.isdir(_p) and _p not in sys.path:
        sys.path.append(_p)

import ml_dtypes

import concourse.bass as bass
import concourse.mybir as mybir
import concourse.tile as tile
from concourse import bacc
from concourse.bass_utils import run_bass_kernel_spmd

P = 128
T = 2048
CIN = 1024
NHC = 4          # heads per core
HS = 64
DOUT = NHC * HS  # 256
NCT = CIN // P   # 8 contraction tiles
NTT = T // P     # 16 t/k tiles
SCALE = 1.0 / 32.0  # 1024 ** -0.5

F32 = mybir.dt.float32
BF16 = mybir.dt.bfloat16
FP16 = mybir.dt.float16


def _build_nc():
    nc = bacc.Bacc("TRN2")

    xc = nc.dram_tensor("xc", [P, NCT * T], BF16, kind="ExternalInput").ap()
    wq = nc.dram_tensor("wq", [P, NCT * DOUT], BF16, kind="ExternalInput").ap()
    wk = nc.dram_tensor("wk", [P, NCT * DOUT], BF16, kind="ExternalInput").ap()
    wv = nc.dram_tensor("wv", [P, NCT * DOUT], BF16, kind="ExternalInput").ap()
    cos4 = nc.dram_tensor("cos4", [P, T], BF16, kind="ExternalInput").ap()
    sin4 = nc.dram_tensor("sin4", [P, T], BF16, kind="ExternalInput").ap()
    utri = nc.dram_tensor("utri", [P, P], BF16, kind="ExternalInput").ap()
    pswap = nc.dram_tensor("pswap", [P, P], BF16, kind="ExternalInput").ap()
    outT = nc.dram_tensor("outT", [DOUT, T], F32, kind="ExternalOutput").ap()

    with tile.TileContext(nc) as tc, ExitStack() as ctx:
        const_pool = ctx.enter_context(tc.tile_pool(name="const", bufs=1))
        wpool = ctx.enter_context(tc.tile_pool(name="w", bufs=1))
        qkpool = ctx.enter_context(tc.tile_pool(name="qk", bufs=1))
        vpool = ctx.enter_context(tc.tile_pool(name="vaug", bufs=1))
        ptpool = ctx.enter_context(tc.tile_pool(name="pt", bufs=1))
        otpool = ctx.enter_context(tc.tile_pool(name="ot", bufs=2))
        rspool = ctx.enter_context(tc.tile_pool(name="rs", bufs=1))
        phase1 = ExitStack()
        xpool = phase1.enter_context(tc.tile_pool(name="x", bufs=1))
        tmppool = phase1.enter_context(tc.tile_pool(name="tmp", bufs=1))

        # ---- inputs to SBUF.  DMAs are issued in consumption order (queue
        # descriptors drain roughly FIFO across the ring): wq + x c-tiles
        # first, then wk, then the late-needed wv / rope / mask constants.
        w_tiles = {}
        for name, wsrc in (("q", wq), ("k", wk), ("v", wv)):
            w_tiles[name] = wpool.tile(
                [P, NCT * DOUT], BF16, tag=f"w{name}", name=f"w{name}"
            )
        xs = xpool.tile([P, NCT * T], BF16, tag="xs")
        cos_s = const_pool.tile([P, T], BF16, tag="cos")
        sin_s = const_pool.tile([P, T], BF16, tag="sin")
        utri_s = const_pool.tile([P, P], BF16, tag="utri")
        pswap_s = const_pool.tile([P, P], BF16, tag="pswap")
        dmas = [(w_tiles["q"][:], wq)]
        dmas += [
            (xs[:, c * T:(c + 1) * T], xc[:, c * T:(c + 1) * T]) for c in range(NCT)
        ]
        dmas.insert(3, (w_tiles["k"][:], wk))
        dmas += [
            (w_tiles["v"][:], wv), (cos_s[:], cos4), (sin_s[:], sin4),
            (pswap_s[:], pswap), (utri_s[:], utri),
        ]
        _engs = (nc.sync, nc.gpsimd, nc.scalar)
        for i, (dst, src) in enumerate(dmas):
            _engs[i % 3].dma_start(dst, src)
        ones64 = const_pool.tile([1, HS], FP16, tag="ones64")
        nc.vector.memset(ones64[:], 1.0)

        # ---- phase 1a: Q^T/K^T projections + RoPE, 512-col chunks.
        # new = ps*cos - swap(ps*sin'); the partition swap runs on the PE as
        # a permutation matmul on the bf16 sin-product, and the subtract on
        # DVE (one PSUM operand).  Swap matmuls trail the projection chunks
        # by one step so the PE never waits on the DVE multiplies.
        qt = [qkpool.tile([P, T], BF16, tag=f"qt{m}", name=f"qt{m}") for m in range(2)]
        kt = [qkpool.tile([P, T], BF16, tag=f"kt{m}", name=f"kt{m}") for m in range(2)]

        with tc.tile_pool(name="pp_proj", bufs=3, space="PSUM") as pp_proj, \
                tc.tile_pool(name="pp_sw", bufs=2, space="PSUM") as pp_sw:
            pending = []

            def flush_swap():
                while pending:
                    dst_sl, apr_p = pending.pop(0)
                    swp = pp_sw.tile([P, 512], F32, tag="swp")
                    nc.tensor.matmul(
                        swp[:], lhsT=pswap_s[:], rhs=apr_p[:],
                        start=True, stop=True,
                    )
                    nc.vector.tensor_sub(dst_sl, aprev.pop(0)[:], swp[:])

            aprev = []
            for wname, dst, m in (
                ("q", qt, 0), ("k", kt, 0), ("q", qt, 1), ("k", kt, 1)
            ):
                w_s = w_tiles[wname]
                for nch in range(4):
                    sl = slice(nch * 512, (nch + 1) * 512)
                    ps = pp_proj.tile([P, 512], F32, tag="proj")
                    for c in range(NCT):
                        nc.tensor.matmul(
                            ps[:],
                            lhsT=w_s[:, c * DOUT + m * P: c * DOUT + (m + 1) * P],
                            rhs=xs[:, c * T + nch * 512: c * T + (nch + 1) * 512],
                            start=(c == 0),
                            stop=(c == NCT - 1),
                        )
                    a = tmppool.tile([P, 512], BF16, tag="ropeA", bufs=3)
                    apr = tmppool.tile([P, 512], BF16, tag="ropeAp", bufs=3)
                    nc.vector.tensor_mul(a[:], ps[:], cos_s[:, sl])
                    nc.vector.tensor_mul(apr[:], ps[:], sin_s[:, sl])
                    flush_swap()
                    aprev.append(a)
                    pending.append((dst[m][:, sl], apr))
            flush_swap()

        # ---- phase 1b/2 shared machinery
        w_v = w_tiles["v"]
        va = []
        pp_s = ctx.enter_context(tc.tile_pool(name="pp_s", bufs=2, space="PSUM"))
        pp_ob = {}  # pp_o/pp_b created after phase1's PSUM pool closes
        pp_v = phase1.enter_context(tc.tile_pool(name="pp_v", bufs=4, space="PSUM"))

        # Pending PE work items (cost_ns, emit_fn).  Score chunks are the
        # pacing stream (exp on Scalar is ~2.5x slower than the score
        # matmuls); between chunks the queue drains PV / broadcast / V-tile
        # matmuls so the in-order PE stream never head-of-line blocks on exp.
        pending = []

        def pump(budget_ns):
            spent = 0
            while pending and spent < budget_ns:
                cost, fn = pending.pop(0)
                fn()
                spent += cost

        def queue_v_tiles():
            for t in range(NTT):
                def fn(t=t):
                    vt = vpool.tile(
                        [P, NHC * (HS + 1)], BF16, tag=f"vaug{t}", name=f"vaug{t}"
                    )
                    vt_r = vt.rearrange("p (h e) -> p h e", e=HS + 1)
                    nc.gpsimd.memset(vt_r[:, :, HS:HS + 1], 1.0)
                    vp = pp_v.tile([P, DOUT], F32, tag="vp", name="vp")
                    for c in range(NCT):
                        nc.tensor.matmul(
                            vp[:],
                            lhsT=xs[:, c * T + t * P: c * T + (t + 1) * P],
                            rhs=w_v[:, c * DOUT:(c + 1) * DOUT],
                            start=(c == 0),
                            stop=(c == NCT - 1),
                        )
                    nc.vector.tensor_copy(
                        vt_r[:, :, 0:HS], vp.rearrange("p (h d) -> p h d", d=HS)
                    )
                    va.append(vt)
                pending.append((880, fn))

        def ensure_ppob():
            if "o" not in pp_ob:
                # all V-tile items have drained (they precede any PV item in
                # the queue), so xs/tmp/pp_v can be released now
                phase1.close()
                pp_ob["o"] = ctx.enter_context(
                    tc.tile_pool(name="pp_o", bufs=2, space="PSUM")
                )
                pp_ob["b"] = ctx.enter_context(
                    tc.tile_pool(name="pp_b", bufs=2, space="PSUM")
                )

        def scores_j(pair, j, pts):
            """Score matmuls + exp + diag mask for k-block j of a pair,
            pumping queued PE work between chunks."""
            qt_t, kt_t = qt[pair[0] // 2], kt[pair[0] // 2]
            w_j = T - j * P
            ptj = ptpool.tile(
                [P, 2 * w_j], BF16, tag=f"pt{j}", name=f"pt{j}",
                bufs=2 if j < 2 else None,
            )
            pts.append(ptj)
            pt_r = ptj.rearrange("p (h w) -> p h w", h=2)
            for s in range(0, w_j, 512):
                n = min(512, w_j - s)
                ps = pp_s.tile([P, 1024], F32, tag="ps", name="ps")
                for hi in range(2):
                    r0 = hi * HS
                    nc.tensor.matmul(
                        ps[:, hi * 512: hi * 512 + n],
                        lhsT=kt_t[r0:r0 + HS, j * P:(j + 1) * P],
                        rhs=qt_t[r0:r0 + HS, j * P + s: j * P + s + n],
                        start=True,
                        stop=True,
                        tile_position=(r0, 0),
                    )
                nc.scalar.activation(
                    pt_r[:, :, s:s + n],
                    ps.rearrange("p (h c) -> p h c", h=2)[:, :, 0:n],
                    mybir.ActivationFunctionType.Exp,
                    scale=SCALE,
                )
                pump(int(0.9 * n) + 150)
            # causal mask on the diagonal block (col 0 = q-offset j*128)
            for hi in range(2):
                nc.vector.tensor_mul(
                    ptj[:, hi * w_j: hi * w_j + P],
                    ptj[:, hi * w_j: hi * w_j + P],
                    utri_s[:],
                )

        def queue_pv_chunk(pair, qc, pts, norm_q):
            """Queue the PV accumulation chains for q-chunk qc (both heads),
            in batches of two k-tiles, followed by the denominator recip."""
            q0 = qc * 512
            jmax = 4 * qc + 3
            st = {}
            order = [jj for jj in range(jmax + 1) if jj * P <= q0]
            order += [jj for jj in range(jmax + 1) if jj * P > q0]
            steps = [
                (jj, i == 0, i == jmax) for i, jj in enumerate(order)
            ]
            for hi, h in enumerate(pair):
                for b0 in range(0, len(steps), 2):
                    batch = steps[b0:b0 + 2]
                    def fn(batch=batch, hi=hi, h=h, first=(b0 == 0)):
                        ensure_ppob()
                        if first:
                            st[hi] = pp_ob["o"].tile(
                                [HS + 1, 512], F32, tag="po", name=f"po{hi}"
                            )
                        po = st[hi]
                        for jj, fst, lst in batch:
                            col0 = max(0, jj * P - q0)
                            w_jj = T - jj * P
                            qoff = q0 + col0 - jj * P
                            nc.tensor.matmul(
                                po[:, col0:512],
                                lhsT=va[jj][:, h * (HS + 1):(h + 1) * (HS + 1)],
                                rhs=pts[jj][
                                    :, hi * w_jj + qoff: hi * w_jj + qoff + 512 - col0
                                ],
                                start=fst,
                                stop=lst,
                                skip_group_check=True,
                            )
                    pending.append((440, fn))

            def fin():
                # denominators for both heads (rows 0 and 64 keep partition
                # bases aligned) -> one reciprocal + two fp16 casts
                dn = rspool.tile([HS + 1, 512], F32, tag="dn", name="dn")
                for hi in range(2):
                    nc.vector.tensor_copy(
                        dn[hi * HS:hi * HS + 1, :], st[hi][HS:HS + 1, :]
                    )
                rs = rspool.tile([HS + 1, 512], F32, tag="rs", name="rs")
                nc.vector.reciprocal_approx_fast(rs[:], dn[:])
                st["rs16"] = []
                for hi in range(2):
                    r16 = rspool.tile(
                        [1, 512], FP16, tag=f"rs16_{hi}", name=f"rs16_{hi}"
                    )
                    nc.vector.tensor_copy(r16[:], rs[hi * HS:hi * HS + 1, :])
                    st["rs16"].append(r16)
            pending.append((0, fin))
            norm_q.append((qc, st))

        def queue_norm(pair, item):
            """Queue the 1/denom broadcast + output write for a chunk."""
            qc, st = item
            q0 = qc * 512
            for hi, h in enumerate(pair):
                def fn(hi=hi, h=h):
                    poS = otpool.tile([HS, 512], F32, tag="poS", name=f"poS{hi}")
                    nc.scalar.activation(
                        poS[:], st[hi][0:HS, :], mybir.ActivationFunctionType.Copy
                    )
                    pb = pp_ob["b"].tile([HS, 512], F32, tag="pb", name=f"pb{hi}")
                    nc.tensor.matmul(
                        pb[:],
                        lhsT=ones64[:],
                        rhs=st["rs16"][hi][:],
                        start=True,
                        stop=True,
                    )
                    ot = otpool.tile([HS, 512], F32, tag="ot", name="ot")
                    nc.vector.tensor_mul(ot[:], poS[:], pb[:])
                    (nc.sync, nc.gpsimd)[hi].dma_start(
                        outT[h * HS:(h + 1) * HS, q0:q0 + 512], ot[:]
                    )
                pending.append((400, fn))

        # ---- phase 2 schedule
        pairs = ((0, 1), (2, 3))
        queue_v_tiles()
        for pi, pair in enumerate(pairs):
            pts = []
            norm_q = []
            for j in range(NTT):
                scores_j(pair, j, pts)
                if j % 4 == 0 and j > 0:
                    queue_pv_chunk(pair, j // 4 - 1, pts, norm_q)
                elif j % 4 == 1 and norm_q:
                    queue_norm(pair, norm_q.pop(0))
            queue_pv_chunk(pair, 3, pts, norm_q)
            queue_norm(pair, norm_q.pop(0))
        pump(1 << 30)
    nc.compile()
    return nc


_CACHE = {}


def _get_nc():
    if "nc" not in _CACHE:
        _CACHE["nc"] = _build_nc()
    return _CACHE["nc"]


def _host_inputs(x, Wq, Wk, Wv):
    bf = ml_dtypes.bfloat16
    # RoPE tables (match reference: theta over hs/2 freqs with dim=n_emb)
    i = np.arange(HS // 2, dtype=np.float32)
    theta = np.float32(10000.0) ** (-2.0 * i / np.float32(CIN))
    pos = np.arange(T, dtype=np.float32)
    ang = pos[:, None] * theta[None, :]
    cosT = np.cos(ang).T.astype(np.float32)  # [32, T]
    sinT = np.sin(ang).T.astype(np.float32)
    cos4 = np.ascontiguousarray(np.tile(cosT, (4, 1))).astype(bf)  # [128, T]
    sin4 = np.ascontiguousarray(
        np.tile(np.concatenate([-sinT, sinT], axis=0), (2, 1))
    ).astype(bf)  # rows: [-sin, +sin] x2
    utri_np = np.triu(np.ones((P, P), np.float32)).astype(bf)
    pswap_np = np.zeros((P, P), np.float32)
    pswap_np[np.arange(P), np.arange(P) ^ 32] = 1.0
    pswap_np = pswap_np.astype(bf)

    def cmajor(w):  # [256 out rows, 1024 in] -> [128, 8*256] c-tile-major
        return np.ascontiguousarray(
            w.T.reshape(NCT, P, DOUT).transpose(1, 0, 2).reshape(P, NCT * DOUT)
        ).astype(bf)

    perm = np.concatenate([np.arange(0, HS, 2), np.arange(1, HS, 2)])
    in_maps = []
    for core in range(8):
        b, g = core // 4, core % 4
        idx = np.concatenate([(4 * g + h) * HS + perm for h in range(NHC)])
        xb = np.ascontiguousarray(
            x[b].T.reshape(NCT, P, T).transpose(1, 0, 2).reshape(P, NCT * T)
        ).astype(bf)
        m = {
            "xc": xb,
            "wq": cmajor(Wq[idx]),
            "wk": cmajor(Wk[idx]),
            "wv": cmajor(Wv[g * DOUT:(g + 1) * DOUT]),
            "cos4": cos4,
            "sin4": sin4,
            "utri": utri_np,
            "pswap": pswap_np,
        }
        in_maps.append(m)
    return in_maps


def kernel(x, Wq, Wk, Wv, _trace=False, _trace_kwargs=None):
    x = np.asarray(x)
    Wq, Wk, Wv = np.asarray(Wq), np.asarray(Wk), np.asarray(Wv)
    B = x.shape[0]
    nc = _get_nc()
    in_maps = _host_inputs(x, Wq, Wk, Wv)
    res = run_bass_kernel_spmd(
        nc, in_maps, list(range(8)), trace=_trace, **(_trace_kwargs or {})
    )
    out = np.zeros((B, T, CIN), np.float32)
    for core in range(8):
        b, g = core // 4, core % 4
        out[b, :, g * DOUT:(g + 1) * DOUT] = res.results[core]["outT"].T
    if _trace:
        return out, res
    return out
